# revision 44
# baseline (speedup 1.0000x reference)
"""Cox proportional-hazards loss (CoxNNet) on 8 Trainium2 NeuronCores.

loss = -mean((theta - log(risk_sum)) * events)
risk_sum[i] = sum_j [d_j >= d_i] * exp(theta_j)        (N = 16384)

Sharding: rows i of the [N, N] risk-set reduction are split across 8 cores
(2048 rows each). Host ships ONLY each core's shard, packed into one
[3*2048] f32 tensor per core ([d | theta | events], 24 KB/core, 192 KB
total over the tunnel); the full d / theta vectors each core needs for
the j-reduction are rebuilt on device with two 8-way DRAM AllGathers
(NeuronLink, ~microseconds) instead of being replicated from the host.

Per core:
  - DMA its packed shard to bounce DRAM, AllGather d and theta to [N],
  - loads d, theta in a [128, 128] chunk layout (partition = fast index),
  - w = exp(theta) on the scalar engine,
  - for each of 128 j-chunks: DVE tensor_scalar(is_le) builds the
    [128-j x 2048-i] 0/1 mask (cols [0,1536)); scalar-engine
    Sign(d_j - d_i) covers cols [1536,2048) (fixed up exactly in the
    epilogue via risk = 0.5*(S + W + w_i)); 4 fp32 matmuls
    (lhsT = w-chunk [128,1]) accumulate risk_sum into 4 [1,512] PSUM rows,
  - epilogue: risk -> ln -> (theta_i - ln) * e_i -> free-dim reduce
    -> one f32 partial per core.
Host combines: loss = -(sum of partials) / N.

Launch path: the jitted shard_map executable is built ONCE per process
and cached; each kernel() call is a single pipelined PJRT dispatch
(host->device shard upload, execute, [8,1] readback) = one tunnel
round trip.
"""

import numpy as np

N = 16384
P = 128
NCH = N // P            # 128 j-chunks per core (all j)
NCORES = 8
NI = N // NCORES        # 2048 i-rows per core
FT = 512                # fp32 moving-operand max / one PSUM bank
NF = NI // FT           # 4 PSUM accumulators
# Column split of the per-chunk mask generation between engines:
# [0, DVE_COLS) via vector tensor_scalar(is_le)  (exact 0/1)
# [DVE_COLS, NI) via scalar-engine Sign(d_j - d_i) (+1/0/-1, fixed up in
# the epilogue: risk = 0.5*(S + W + w_i); exact except d_j==d_i ties j!=i)
# NTFF-measured producer rates: DVE ~0.73 ns/col (2x_2p mode), ACT Sign
# ~1.11 ns/col. 1280/768 balances both at ~900 ns/chunk, just under the
# PE's ~980 ns/chunk consumption, so the tensor engine never starves.
DVE_COLS = 1280
SH3 = 3 * NI            # packed per-core input [d | theta | events]

_CACHE = {}


def _build():
    import concourse.bacc as bacc
    import concourse.mybir as mybir
    from concourse.tile import TileContext

    F32 = mybir.dt.float32
    BF16 = mybir.dt.bfloat16
    act_cols = NI - DVE_COLS
    dve_cols = DVE_COLS

    nc = bacc.Bacc(num_devices=NCORES)
    packed = nc.declare_dram_parameter("packed", [SH3], F32, isOutput=False)
    out = nc.declare_dram_parameter("partial", [1, 1], F32, isOutput=True)
    # collectives can't touch I/O tensors directly -> bounce through
    # internal DRAM. d and theta ride ONE AllGather (halves the ~us-scale
    # collective launch latency on the startup critical path): bounce row 0
    # is the d shard, row 1 the theta shard, so the gathered flat buffer is
    # [k, t, q, c] = core k, d/theta t, 16 chunks q of 128 elements c.
    bounce2 = nc.dram_tensor("bounce2", [2, NI], F32)
    gath2 = nc.dram_tensor("gath2", [2 * N], F32)
    # epilogue bounce: risk_row [1, NI] -> [128, NI/128] partition-spread
    scratch = nc.dram_tensor("scratch", [NI], F32)

    with TileContext(nc) as tc:
        with (
            tc.tile_pool(name="const", bufs=1) as cpool,
            tc.tile_pool(name="mask", bufs=6) as mpool,
            tc.tile_pool(name="acc", bufs=1, space="PSUM") as ppool,
            tc.tile_pool(name="bc", bufs=1, space="PSUM") as bcpool,
        ):
            sb_d = cpool.tile([P, NCH], F32)    # d[p*128 + c] at [p, c]
            sb_th = cpool.tile([P, NCH], F32)
            w_act = cpool.tile([P, NCH], F32)   # exp(theta), ACT-written
            w_mm = cpool.tile([P, NCH], BF16)   # bf16 w for PE (1 cyc/row)
            w_sb = cpool.tile([P, NCH], F32)    # fp32 copy of the ROUNDED w
            ones_row = cpool.tile([1, P], F32)  # bcast lhsT [K=1, M=128]
            row_di = cpool.tile([1, NI], F32)
            row_di2 = cpool.tile([1, NI], F32)  # DVE copy of row_di
            row_thi = cpool.tile([1, NI], F32)
            bc_di = cpool.tile([P, NI], F32)
            risk_row = cpool.tile([1, NI], F32)
            part_sb = cpool.tile([1, 1], F32)

            # ---- rebuild full d / theta on device (one AllGather) ----
            nc.sync.dma_start(out=bounce2[0:1, :],
                              in_=packed[0:NI].rearrange("(o n) -> o n", o=1))
            nc.sync.dma_start(out=bounce2[1:2, :],
                              in_=packed[NI:2 * NI].rearrange("(o n) -> o n", o=1))
            nc.gpsimd.collective_compute(
                "AllGather", mybir.AluOpType.bypass,
                replica_groups=[list(range(NCORES))],
                ins=[bounce2[:, :].opt()], outs=[gath2[:].opt()],
            )

            # ---- loads ----
            # gathered flat index = k*2*NI + t*NI + q*128 + c ; the (p c)
            # SBUF chunk layout wants d[p*128+c] at [p, c] with p = k*16+q.
            # DMA pairs in/out APs in flattened element order, so the 4-D
            # source view (k, t-slice, q, c) lands on partitions p = k*16+q.
            g4 = gath2[:].rearrange("(k t q c) -> k t q c",
                                    k=NCORES, t=2, q=P // NCORES)
            nc.sync.dma_start(out=sb_d[:, :], in_=g4[:, 0:1, :, :])
            nc.sync.dma_start(out=sb_th[:, :], in_=g4[:, 1:2, :, :])
            nc.sync.dma_start(out=row_di[:, :],
                              in_=packed[0:NI].rearrange("(o n) -> o n", o=1))
            nc.sync.dma_start(out=row_thi[:, :],
                              in_=packed[NI:2 * NI].rearrange("(o n) -> o n", o=1))

            # ---- prep ----
            # All PE operands run in bf16 (1 PE cycle/moving row vs fp32's 4;
            # mask values 0/1 are exact in bf16, only w rounds ~2^-9 relative,
            # and the PSUM accumulate stays fp32). w_sb is the fp32 image of
            # the ROUNDED w so the sign-fixup identity
            # risk = 0.5*(S + W + w_i) uses the same rounded values as S.
            nc.scalar.activation(w_act[:, :], sb_th[:, :],
                                 mybir.ActivationFunctionType.Exp)
            nc.vector.tensor_copy(w_mm[:, :], w_act[:, :])   # round to bf16
            nc.vector.tensor_copy(w_sb[:, :], w_mm[:, :])    # rounded, fp32
            nc.vector.memset(ones_row[:, :], 1.0)
            nc.vector.tensor_copy(row_di2[:, :], row_di[:, :])
            for t in range(NF):
                bc_ps = bcpool.tile([P, FT], F32, tag="bc")
                nc.tensor.matmul(
                    bc_ps[:, :], lhsT=ones_row[:, :],
                    rhs=row_di2[:, t * FT:(t + 1) * FT], start=True, stop=True,
                )
                nc.vector.tensor_copy(bc_di[:, t * FT:(t + 1) * FT], bc_ps[:, :])

            # ---- main loop: mask gen + masked reduce ----
            # matmul segments: [0, NI) cut at PSUM-bank-size boundaries (FT)
            # and at the DVE/ACT source boundary (dve_cols). Each segment
            # accumulates into its OWN PSUM tile — two interleaved start/stop
            # chains must not share a bank (bf16 runs 1 cyc/row at any size).
            bounds = sorted({0, NI, dve_cols} | {t * FT for t in range(1, NF)})
            segs = [(lo, hi, ppool.tile([1, hi - lo], F32, name=f"risk{lo}"))
                    for lo, hi in zip(bounds, bounds[1:])]
            for c in range(NCH):
                # separate tiles per producing engine — a shared tile would
                # WAW-serialize DVE behind ACT in the Tile dep tracker
                mask_d = None
                mask_a = None
                if dve_cols > 0:
                    mask_d = mpool.tile([P, dve_cols], BF16, tag="mask_d",
                                        name=f"mask_d{c}")
                if act_cols > 0:
                    mask_a = mpool.tile([P, act_cols], BF16, tag="mask_a",
                                        name=f"mask_a{c}")
                if mask_d is not None:
                    nc.vector.tensor_scalar(
                        mask_d[:, :], bc_di[:, :dve_cols],
                        sb_d[:, c:c + 1], None, mybir.AluOpType.is_le,
                    )
                if mask_a is not None:
                    # Sign(d_j - d_i) = Sign(-1.0 * d_i + bias d_j)
                    nc.scalar.activation(
                        mask_a[:, :], bc_di[:, dve_cols:],
                        mybir.ActivationFunctionType.Sign,
                        bias=sb_d[:, c:c + 1], scale=-1.0,
                    )
                for lo, hi, ps in segs:
                    if hi <= dve_cols:
                        rhs = mask_d[:, lo:hi]
                    else:
                        rhs = mask_a[:, lo - dve_cols:hi - dve_cols]
                    nc.tensor.matmul(
                        ps[:, :], lhsT=w_mm[:, c:c + 1],
                        rhs=rhs,
                        start=(c == 0), stop=(c == NCH - 1),
                    )

            # ---- epilogue ----
            for lo, hi, ps in segs:
                nc.vector.tensor_copy(risk_row[:, lo:hi], ps[:, :])

            if act_cols > 0:
                # risk = 0.5 * (S + W + w_i) for sign-generated columns
                ones_col = cpool.tile([P, 1], F32)
                wsum_p = cpool.tile([P, 1], F32)
                w_row_bf = cpool.tile([1, NI], BF16)
                w_row = cpool.tile([1, NI], F32)
                wtot_ps = bcpool.tile([1, 1], F32, tag="wtot")
                wtot_sb = cpool.tile([1, 1], F32)
                nc.vector.memset(ones_col[:, :], 1.0)
                nc.vector.reduce_sum(wsum_p[:, :], w_sb[:, :],
                                     axis=mybir.AxisListType.X)
                nc.tensor.matmul(wtot_ps[:, :], lhsT=wsum_p[:, :],
                                 rhs=ones_col[:, :], start=True, stop=True)
                nc.vector.tensor_copy(wtot_sb[:, :], wtot_ps[:, :])
                # w_i must round through bf16 exactly like the w the PE saw
                nc.scalar.activation(w_row_bf[:, :], row_thi[:, :],
                                     mybir.ActivationFunctionType.Exp)
                nc.vector.tensor_copy(w_row[:, :], w_row_bf[:, :])
                a0 = dve_cols
                sw_half = cpool.tile([1, NI], F32)
                # (S + W) * 0.5
                nc.vector.tensor_scalar(
                    sw_half[:, a0:], risk_row[:, a0:], wtot_sb[:, :], 0.5,
                    mybir.AluOpType.add, mybir.AluOpType.mult,
                )
                # risk = w_i * 0.5 + (S + W) * 0.5
                nc.vector.scalar_tensor_tensor(
                    risk_row[:, a0:], w_row[:, a0:], 0.5, sw_half[:, a0:],
                    mybir.AluOpType.mult, mybir.AluOpType.add,
                )

            # The ln/sub/mul/reduce chain on the single-partition [1, NI] row
            # costs ~8.7us (one DVE/ACT lane). Spread risk over all 128
            # partitions via a DRAM bounce ([1, NI] -> [128, NI/128] with
            # r128[p, q] = risk[q*128 + p]); theta_i and e_i load directly in
            # that layout from the packed input. Same math, ~128 lanes wide.
            QW = NI // P  # 16
            r128 = cpool.tile([P, QW], F32)
            th128 = cpool.tile([P, QW], F32)
            e128 = cpool.tile([P, QW], F32)
            ln128 = cpool.tile([P, QW], F32)
            diff128 = cpool.tile([P, QW], F32)
            prod128 = cpool.tile([P, QW], F32)
            psum_p = cpool.tile([P, 1], F32)
            part_ps = bcpool.tile([1, 1], F32, tag="part")
            nc.sync.dma_start(out=scratch[:], in_=risk_row[:, :])
            nc.sync.dma_start(out=r128[:, :],
                              in_=scratch[:].rearrange("(q p) -> p q", p=P))
            nc.sync.dma_start(out=th128[:, :],
                              in_=packed[NI:2 * NI].rearrange("(q p) -> p q", p=P))
            nc.sync.dma_start(out=e128[:, :],
                              in_=packed[2 * NI:3 * NI].rearrange("(q p) -> p q", p=P))
            # (tensor_tensor_reduce crashes at runtime on this stack — use
            # separate mul + reduce_sum instead)
            nc.scalar.activation(ln128[:, :], r128[:, :],
                                 mybir.ActivationFunctionType.Ln)
            nc.vector.tensor_sub(diff128[:, :], th128[:, :], ln128[:, :])
            nc.vector.tensor_mul(prod128[:, :], diff128[:, :], e128[:, :])
            nc.vector.reduce_sum(psum_p[:, :], prod128[:, :],
                                 axis=mybir.AxisListType.X)
            # cross-partition total: ones-matmul [128,1] x [128,1] -> [1,1]
            nc.tensor.matmul(part_ps[:, :], lhsT=psum_p[:, :],
                             rhs=ones_col[:, :], start=True, stop=True)
            nc.vector.tensor_copy(part_sb[:, :], part_ps[:, :])
            nc.sync.dma_start(out=out[:, :], in_=part_sb[:, :])

    nc.finalize()
    return nc


def _get_nc():
    if "nc" not in _CACHE:
        _CACHE["nc"] = _build()
    return _CACHE["nc"]


def _get_fn():
    """Build (once) a cached compiled shard_map executable for the kernel.

    Mirrors concourse.bass2jax.run_bass_via_pjrt's multi-core path with
    three launch-overhead cuts: the compiled callable is kept alive
    across kernel() calls (no re-trace/re-lower/executable reload), the
    donated zero-output operands are dropped (the kernel fully writes
    its [1,1] output, so uninitialized PJRT result buffers are fine),
    and the executable is compiled under fast_dispatch (no bass_effect
    -> C++ fast-path dispatch). Each call is then a single pipelined
    tunnel round trip.
    """
    if "fn" in _CACHE:
        return _CACHE["fn"]

    import jax
    from jax.sharding import Mesh, NamedSharding, PartitionSpec
    try:
        from jax.experimental.shard_map import shard_map
    except ImportError:  # newer jax
        from jax import shard_map
    import concourse.bass2jax as b2j
    import concourse.mybir as mybir

    nc = _get_nc()
    b2j.install_neuronx_cc_hook()
    partition_name = (nc.partition_id_tensor.name
                      if nc.partition_id_tensor else None)

    in_names = []
    out_names = []
    out_avals = []
    for alloc in nc.m.functions[0].allocations:
        if not isinstance(alloc, mybir.MemoryLocationSet):
            continue
        name = alloc.memorylocations[0].name
        if alloc.kind == "ExternalInput":
            if name != partition_name:
                in_names.append(name)
        elif alloc.kind == "ExternalOutput":
            out_names.append(name)
            shape = tuple(alloc.tensor_shape)
            dtype = mybir.dt.np(alloc.dtype)
            out_avals.append(jax.core.ShapedArray(shape, dtype))
    all_in_names = (list(in_names)
                    + ([partition_name] if partition_name else []))

    def _body(*args):
        operands = list(args)
        if partition_name is not None:
            operands.append(b2j.partition_id_tensor())
        outs = b2j._bass_exec_p.bind(
            *operands,
            out_avals=tuple(out_avals),
            in_names=tuple(all_in_names),
            out_names=tuple(out_names),
            lowering_input_output_aliases=(),
            sim_require_finite=True,
            sim_require_nnan=True,
            nc=nc,
        )
        return tuple(outs)

    devices = jax.devices()[:NCORES]
    assert len(devices) == NCORES, f"need {NCORES} devices, have {len(devices)}"
    mesh = Mesh(np.asarray(devices), ("core",))
    in_specs = (PartitionSpec("core"),) * len(in_names)
    out_specs = (PartitionSpec("core"),) * len(out_names)
    sharded = shard_map(_body, mesh=mesh, in_specs=in_specs,
                        out_specs=out_specs, check_rep=False)
    in_sharding = NamedSharding(mesh, PartitionSpec("core"))
    arg_structs = [jax.ShapeDtypeStruct((NCORES * SH3,), np.float32,
                                        sharding=in_sharding)]
    try:
        fn = b2j.fast_dispatch_compile(
            lambda: jax.jit(sharded, keep_unused=True)
            .lower(*arg_structs).compile()
        )
    except Exception:
        fn = jax.jit(sharded, keep_unused=True)
    _CACHE["fn"] = fn
    return _CACHE["fn"]


def _pack_inputs(hazard_pred, durations, events):
    # If the harness hands us device-resident jax arrays, start all three
    # host copies concurrently before the (blocking) np.asarray fetches.
    for x in (hazard_pred, durations, events):
        if hasattr(x, "copy_to_host_async"):
            try:
                x.copy_to_host_async()
            except Exception:
                pass
    theta = np.asarray(hazard_pred, dtype=np.float32).reshape(-1)
    d = np.asarray(durations, dtype=np.float32).reshape(-1)
    e = np.asarray(events, dtype=np.float32).reshape(-1)
    packed = np.empty(NCORES * SH3, np.float32)
    pv = packed.reshape(NCORES, 3, NI)
    pv[:, 0, :] = d.reshape(NCORES, NI)
    pv[:, 1, :] = theta.reshape(NCORES, NI)
    pv[:, 2, :] = e.reshape(NCORES, NI)
    return packed, pv


def _run_fast(packed):
    fn = _get_fn()
    out_arrs = fn(packed)
    return np.asarray(out_arrs[0]).reshape(NCORES, -1)[:, 0]


def _run_fallback(pv):
    from concourse.bass_utils import run_bass_kernel_spmd
    nc = _get_nc()
    in_maps = [{"packed": np.ascontiguousarray(pv[c].reshape(-1))}
               for c in range(NCORES)]
    res = run_bass_kernel_spmd(nc, in_maps, core_ids=list(range(NCORES)),
                               trace=False)
    return np.asarray([np.asarray(r["partial"]).reshape(-1)[0]
                       for r in res.results])


def kernel(hazard_pred, durations, events):
    packed, pv = _pack_inputs(hazard_pred, durations, events)
    try:
        partials = _run_fast(packed)
    except Exception:
        partials = _run_fallback(pv)
    loss = -(np.sum(np.asarray(partials, dtype=np.float64)) / N)
    return np.asarray(loss, dtype=np.float32)


# revision 45
# speedup vs baseline: 1.1844x; 1.1844x over previous
"""Cox proportional-hazards loss (CoxNNet) on 8 Trainium2 NeuronCores.

loss = -mean((theta - log(risk_sum)) * events)
risk_sum[i] = sum_j [d_j >= d_i] * exp(theta_j)        (N = 16384)

Sharding: rows i of the [N, N] risk-set reduction are split across 8 cores
(2048 rows each). Host ships ONLY each core's shard, packed into one
[3*2048] f32 tensor per core ([d | theta | events], 24 KB/core, 192 KB
total over the tunnel); the full d / theta vectors each core needs for
the j-reduction are rebuilt on device with two 8-way DRAM AllGathers
(NeuronLink, ~microseconds) instead of being replicated from the host.

Per core:
  - DMA its packed shard to bounce DRAM, AllGather d and theta to [N],
  - loads d, theta in a [128, 128] chunk layout (partition = fast index),
  - w = exp(theta) on the scalar engine,
  - for each of 128 j-chunks: DVE tensor_scalar(is_le) builds the
    [128-j x 2048-i] 0/1 mask (cols [0,1536)); scalar-engine
    Sign(d_j - d_i) covers cols [1536,2048) (fixed up exactly in the
    epilogue via risk = 0.5*(S + W + w_i)); 4 fp32 matmuls
    (lhsT = w-chunk [128,1]) accumulate risk_sum into 4 [1,512] PSUM rows,
  - epilogue: risk -> ln -> (theta_i - ln) * e_i -> free-dim reduce
    -> one f32 partial per core.
Host combines: loss = -(sum of partials) / N.

Launch path: the jitted shard_map executable is built ONCE per process
and cached; each kernel() call is a single pipelined PJRT dispatch
(host->device shard upload, execute, [8,1] readback) = one tunnel
round trip.
"""

import numpy as np

N = 16384
P = 128
NCH = N // P            # 128 j-chunks per core (all j)
NCORES = 8
NI = N // NCORES        # 2048 i-rows per core
FT = 512                # fp32 moving-operand max / one PSUM bank
NF = NI // FT           # 4 PSUM accumulators
# Column split of the per-chunk mask generation between engines:
# [0, DVE_COLS) via vector tensor_scalar(is_le)  (exact 0/1)
# [DVE_COLS, NI) via scalar-engine Sign(d_j - d_i) (+1/0/-1, fixed up in
# the epilogue: risk = 0.5*(S + W + w_i); exact except d_j==d_i ties j!=i)
# NTFF-measured producer rates: DVE ~0.73 ns/col (2x_2p mode), ACT Sign
# ~1.11 ns/col. 1280/768 balances both at ~900 ns/chunk, just under the
# PE's ~980 ns/chunk consumption, so the tensor engine never starves.
DVE_COLS = 1280
SH3 = 3 * NI            # packed per-core input [d | theta | events]

_CACHE = {}


def _build():
    import concourse.bacc as bacc
    import concourse.mybir as mybir
    from concourse.tile import TileContext

    F32 = mybir.dt.float32
    BF16 = mybir.dt.bfloat16
    act_cols = NI - DVE_COLS
    dve_cols = DVE_COLS

    nc = bacc.Bacc(num_devices=NCORES)
    packed = nc.declare_dram_parameter("packed", [SH3], F32, isOutput=False)
    out = nc.declare_dram_parameter("partial", [1, 1], F32, isOutput=True)
    # collectives can't touch I/O tensors directly -> bounce through
    # internal DRAM. d and theta ride ONE AllGather (halves the ~us-scale
    # collective launch latency on the startup critical path): bounce row 0
    # is the d shard, row 1 the theta shard, so the gathered flat buffer is
    # [k, t, q, c] = core k, d/theta t, 16 chunks q of 128 elements c.
    bounce2 = nc.dram_tensor("bounce2", [2, NI], F32)
    gath2 = nc.dram_tensor("gath2", [2 * N], F32)
    # epilogue bounce: risk_row [1, NI] -> [128, NI/128] partition-spread
    scratch = nc.dram_tensor("scratch", [NI], F32)

    with TileContext(nc) as tc:
        with (
            tc.tile_pool(name="const", bufs=1) as cpool,
            tc.tile_pool(name="mask", bufs=6) as mpool,
            tc.tile_pool(name="acc", bufs=1, space="PSUM") as ppool,
            tc.tile_pool(name="bc", bufs=1, space="PSUM") as bcpool,
        ):
            sb_d = cpool.tile([P, NCH], F32)    # d[p*128 + c] at [p, c]
            sb_th = cpool.tile([P, NCH], F32)
            w_act = cpool.tile([P, NCH], F32)   # exp(theta), ACT-written
            w_mm = cpool.tile([P, NCH], BF16)   # bf16 w for PE (1 cyc/row)
            w_sb = cpool.tile([P, NCH], F32)    # fp32 copy of the ROUNDED w
            ones_row = cpool.tile([1, P], F32)  # bcast lhsT [K=1, M=128]
            row_di = cpool.tile([1, NI], F32)
            row_di2 = cpool.tile([1, NI], F32)  # DVE copy of row_di
            row_thi = cpool.tile([1, NI], F32)
            bc_di = cpool.tile([P, NI], F32)
            risk_row = cpool.tile([1, NI], F32)
            part_sb = cpool.tile([1, 1], F32)

            # ---- rebuild full d / theta on device (one AllGather) ----
            nc.sync.dma_start(out=bounce2[0:1, :],
                              in_=packed[0:NI].rearrange("(o n) -> o n", o=1))
            nc.sync.dma_start(out=bounce2[1:2, :],
                              in_=packed[NI:2 * NI].rearrange("(o n) -> o n", o=1))
            nc.gpsimd.collective_compute(
                "AllGather", mybir.AluOpType.bypass,
                replica_groups=[list(range(NCORES))],
                ins=[bounce2[:, :].opt()], outs=[gath2[:].opt()],
            )

            # ---- loads ----
            # gathered flat index = k*2*NI + t*NI + q*128 + c ; the (p c)
            # SBUF chunk layout wants d[p*128+c] at [p, c] with p = k*16+q.
            # DMA pairs in/out APs in flattened element order, so the 4-D
            # source view (k, t-slice, q, c) lands on partitions p = k*16+q.
            g4 = gath2[:].rearrange("(k t q c) -> k t q c",
                                    k=NCORES, t=2, q=P // NCORES)
            nc.sync.dma_start(out=sb_d[:, :], in_=g4[:, 0:1, :, :])
            nc.sync.dma_start(out=sb_th[:, :], in_=g4[:, 1:2, :, :])
            nc.sync.dma_start(out=row_di[:, :],
                              in_=packed[0:NI].rearrange("(o n) -> o n", o=1))
            nc.sync.dma_start(out=row_thi[:, :],
                              in_=packed[NI:2 * NI].rearrange("(o n) -> o n", o=1))

            # ---- prep ----
            # All PE operands run in bf16 (1 PE cycle/moving row vs fp32's 4;
            # mask values 0/1 are exact in bf16, only w rounds ~2^-9 relative,
            # and the PSUM accumulate stays fp32). w_sb is the fp32 image of
            # the ROUNDED w so the sign-fixup identity
            # risk = 0.5*(S + W + w_i) uses the same rounded values as S.
            nc.scalar.activation(w_act[:, :], sb_th[:, :],
                                 mybir.ActivationFunctionType.Exp)
            nc.vector.tensor_copy(w_mm[:, :], w_act[:, :])   # round to bf16
            nc.vector.tensor_copy(w_sb[:, :], w_mm[:, :])    # rounded, fp32
            nc.vector.memset(ones_row[:, :], 1.0)
            nc.vector.tensor_copy(row_di2[:, :], row_di[:, :])
            for t in range(NF):
                bc_ps = bcpool.tile([P, FT], F32, tag="bc")
                nc.tensor.matmul(
                    bc_ps[:, :], lhsT=ones_row[:, :],
                    rhs=row_di2[:, t * FT:(t + 1) * FT], start=True, stop=True,
                )
                nc.vector.tensor_copy(bc_di[:, t * FT:(t + 1) * FT], bc_ps[:, :])

            # ---- main loop: mask gen + masked reduce ----
            # matmul segments: [0, NI) cut at PSUM-bank-size boundaries (FT)
            # and at the DVE/ACT source boundary (dve_cols). Each segment
            # accumulates into its OWN PSUM tile — two interleaved start/stop
            # chains must not share a bank (bf16 runs 1 cyc/row at any size).
            # ONE shared mask tile per chunk: DVE writes [0, dve_cols), ACT
            # writes [dve_cols, NI) — disjoint ranges, so the byte-granular
            # dep tracker runs the producers concurrently — and the matmul
            # segments stay FT-aligned (4 per chunk, one fewer dispatch than
            # the split-tile layout; the [1024,1536) segment simply reads
            # across the producer boundary).
            segs = [(t * FT, (t + 1) * FT,
                     ppool.tile([1, FT], F32, name=f"risk{t}"))
                    for t in range(NF)]
            for c in range(NCH):
                mask = mpool.tile([P, NI], BF16, tag="mask", name=f"mask{c}")
                nc.vector.tensor_scalar(
                    mask[:, :dve_cols], bc_di[:, :dve_cols],
                    sb_d[:, c:c + 1], None, mybir.AluOpType.is_le,
                )
                # Sign(d_j - d_i) = Sign(-1.0 * d_i + bias d_j)
                nc.scalar.activation(
                    mask[:, dve_cols:], bc_di[:, dve_cols:],
                    mybir.ActivationFunctionType.Sign,
                    bias=sb_d[:, c:c + 1], scale=-1.0,
                )
                for lo, hi, ps in segs:
                    nc.tensor.matmul(
                        ps[:, :], lhsT=w_mm[:, c:c + 1],
                        rhs=mask[:, lo:hi],
                        start=(c == 0), stop=(c == NCH - 1),
                    )

            # ---- epilogue ----
            for lo, hi, ps in segs:
                nc.vector.tensor_copy(risk_row[:, lo:hi], ps[:, :])

            if act_cols > 0:
                # risk = 0.5 * (S + W + w_i) for sign-generated columns
                ones_col = cpool.tile([P, 1], F32)
                wsum_p = cpool.tile([P, 1], F32)
                w_row_bf = cpool.tile([1, NI], BF16)
                w_row = cpool.tile([1, NI], F32)
                wtot_ps = bcpool.tile([1, 1], F32, tag="wtot")
                wtot_sb = cpool.tile([1, 1], F32)
                nc.vector.memset(ones_col[:, :], 1.0)
                nc.vector.reduce_sum(wsum_p[:, :], w_sb[:, :],
                                     axis=mybir.AxisListType.X)
                nc.tensor.matmul(wtot_ps[:, :], lhsT=wsum_p[:, :],
                                 rhs=ones_col[:, :], start=True, stop=True)
                nc.vector.tensor_copy(wtot_sb[:, :], wtot_ps[:, :])
                # w_i must round through bf16 exactly like the w the PE saw
                nc.scalar.activation(w_row_bf[:, :], row_thi[:, :],
                                     mybir.ActivationFunctionType.Exp)
                nc.vector.tensor_copy(w_row[:, :], w_row_bf[:, :])
                a0 = dve_cols
                sw_half = cpool.tile([1, NI], F32)
                # (S + W) * 0.5
                nc.vector.tensor_scalar(
                    sw_half[:, a0:], risk_row[:, a0:], wtot_sb[:, :], 0.5,
                    mybir.AluOpType.add, mybir.AluOpType.mult,
                )
                # risk = w_i * 0.5 + (S + W) * 0.5
                nc.vector.scalar_tensor_tensor(
                    risk_row[:, a0:], w_row[:, a0:], 0.5, sw_half[:, a0:],
                    mybir.AluOpType.mult, mybir.AluOpType.add,
                )

            # The ln/sub/mul/reduce chain on the single-partition [1, NI] row
            # costs ~8.7us (one DVE/ACT lane). Spread risk over all 128
            # partitions via a DRAM bounce ([1, NI] -> [128, NI/128] with
            # r128[p, q] = risk[q*128 + p]); theta_i and e_i load directly in
            # that layout from the packed input. Same math, ~128 lanes wide.
            QW = NI // P  # 16
            r128 = cpool.tile([P, QW], F32)
            th128 = cpool.tile([P, QW], F32)
            e128 = cpool.tile([P, QW], F32)
            ln128 = cpool.tile([P, QW], F32)
            diff128 = cpool.tile([P, QW], F32)
            prod128 = cpool.tile([P, QW], F32)
            psum_p = cpool.tile([P, 1], F32)
            part_ps = bcpool.tile([1, 1], F32, tag="part")
            nc.sync.dma_start(out=scratch[:], in_=risk_row[:, :])
            nc.sync.dma_start(out=r128[:, :],
                              in_=scratch[:].rearrange("(q p) -> p q", p=P))
            nc.sync.dma_start(out=th128[:, :],
                              in_=packed[NI:2 * NI].rearrange("(q p) -> p q", p=P))
            nc.sync.dma_start(out=e128[:, :],
                              in_=packed[2 * NI:3 * NI].rearrange("(q p) -> p q", p=P))
            # (tensor_tensor_reduce crashes at runtime on this stack — use
            # separate mul + reduce_sum instead)
            nc.scalar.activation(ln128[:, :], r128[:, :],
                                 mybir.ActivationFunctionType.Ln)
            nc.vector.tensor_sub(diff128[:, :], th128[:, :], ln128[:, :])
            nc.vector.tensor_mul(prod128[:, :], diff128[:, :], e128[:, :])
            nc.vector.reduce_sum(psum_p[:, :], prod128[:, :],
                                 axis=mybir.AxisListType.X)
            # cross-partition total: ones-matmul [128,1] x [128,1] -> [1,1]
            nc.tensor.matmul(part_ps[:, :], lhsT=psum_p[:, :],
                             rhs=ones_col[:, :], start=True, stop=True)
            nc.vector.tensor_copy(part_sb[:, :], part_ps[:, :])
            nc.sync.dma_start(out=out[:, :], in_=part_sb[:, :])

    nc.finalize()
    return nc


def _get_nc():
    if "nc" not in _CACHE:
        _CACHE["nc"] = _build()
    return _CACHE["nc"]


def _get_fn():
    """Build (once) a cached compiled shard_map executable for the kernel.

    Mirrors concourse.bass2jax.run_bass_via_pjrt's multi-core path with
    three launch-overhead cuts: the compiled callable is kept alive
    across kernel() calls (no re-trace/re-lower/executable reload), the
    donated zero-output operands are dropped (the kernel fully writes
    its [1,1] output, so uninitialized PJRT result buffers are fine),
    and the executable is compiled under fast_dispatch (no bass_effect
    -> C++ fast-path dispatch). Each call is then a single pipelined
    tunnel round trip.
    """
    if "fn" in _CACHE:
        return _CACHE["fn"]

    import jax
    from jax.sharding import Mesh, NamedSharding, PartitionSpec
    try:
        from jax.experimental.shard_map import shard_map
    except ImportError:  # newer jax
        from jax import shard_map
    import concourse.bass2jax as b2j
    import concourse.mybir as mybir

    nc = _get_nc()
    b2j.install_neuronx_cc_hook()
    partition_name = (nc.partition_id_tensor.name
                      if nc.partition_id_tensor else None)

    in_names = []
    out_names = []
    out_avals = []
    for alloc in nc.m.functions[0].allocations:
        if not isinstance(alloc, mybir.MemoryLocationSet):
            continue
        name = alloc.memorylocations[0].name
        if alloc.kind == "ExternalInput":
            if name != partition_name:
                in_names.append(name)
        elif alloc.kind == "ExternalOutput":
            out_names.append(name)
            shape = tuple(alloc.tensor_shape)
            dtype = mybir.dt.np(alloc.dtype)
            out_avals.append(jax.core.ShapedArray(shape, dtype))
    all_in_names = (list(in_names)
                    + ([partition_name] if partition_name else []))

    def _body(*args):
        operands = list(args)
        if partition_name is not None:
            operands.append(b2j.partition_id_tensor())
        outs = b2j._bass_exec_p.bind(
            *operands,
            out_avals=tuple(out_avals),
            in_names=tuple(all_in_names),
            out_names=tuple(out_names),
            lowering_input_output_aliases=(),
            sim_require_finite=True,
            sim_require_nnan=True,
            nc=nc,
        )
        return tuple(outs)

    devices = jax.devices()[:NCORES]
    assert len(devices) == NCORES, f"need {NCORES} devices, have {len(devices)}"
    mesh = Mesh(np.asarray(devices), ("core",))
    in_specs = (PartitionSpec("core"),) * len(in_names)
    out_specs = (PartitionSpec("core"),) * len(out_names)
    sharded = shard_map(_body, mesh=mesh, in_specs=in_specs,
                        out_specs=out_specs, check_rep=False)
    in_sharding = NamedSharding(mesh, PartitionSpec("core"))
    arg_structs = [jax.ShapeDtypeStruct((NCORES * SH3,), np.float32,
                                        sharding=in_sharding)]
    try:
        fn = b2j.fast_dispatch_compile(
            lambda: jax.jit(sharded, keep_unused=True)
            .lower(*arg_structs).compile()
        )
    except Exception:
        fn = jax.jit(sharded, keep_unused=True)
    _CACHE["fn"] = fn
    return _CACHE["fn"]


def _pack_inputs(hazard_pred, durations, events):
    # If the harness hands us device-resident jax arrays, start all three
    # host copies concurrently before the (blocking) np.asarray fetches.
    for x in (hazard_pred, durations, events):
        if hasattr(x, "copy_to_host_async"):
            try:
                x.copy_to_host_async()
            except Exception:
                pass
    theta = np.asarray(hazard_pred, dtype=np.float32).reshape(-1)
    d = np.asarray(durations, dtype=np.float32).reshape(-1)
    e = np.asarray(events, dtype=np.float32).reshape(-1)
    packed = np.empty(NCORES * SH3, np.float32)
    pv = packed.reshape(NCORES, 3, NI)
    pv[:, 0, :] = d.reshape(NCORES, NI)
    pv[:, 1, :] = theta.reshape(NCORES, NI)
    pv[:, 2, :] = e.reshape(NCORES, NI)
    return packed, pv


def _run_fast(packed):
    fn = _get_fn()
    out_arrs = fn(packed)
    return np.asarray(out_arrs[0]).reshape(NCORES, -1)[:, 0]


def _run_fallback(pv):
    from concourse.bass_utils import run_bass_kernel_spmd
    nc = _get_nc()
    in_maps = [{"packed": np.ascontiguousarray(pv[c].reshape(-1))}
               for c in range(NCORES)]
    res = run_bass_kernel_spmd(nc, in_maps, core_ids=list(range(NCORES)),
                               trace=False)
    return np.asarray([np.asarray(r["partial"]).reshape(-1)[0]
                       for r in res.results])


def kernel(hazard_pred, durations, events):
    packed, pv = _pack_inputs(hazard_pred, durations, events)
    try:
        partials = _run_fast(packed)
    except Exception:
        partials = _run_fallback(pv)
    loss = -(np.sum(np.asarray(partials, dtype=np.float64)) / N)
    return np.asarray(loss, dtype=np.float32)
